# revision 10
# baseline (speedup 1.0000x reference)
"""MiniSTU Trainium2 kernel (8 NeuronCores, Bass/Tile).

Math: the reference's FFT convolution + einsum collapses to
    y[b,l,o] = sum_g sum_{t<=l} phi_eff_g[l-t] * (x[b,t] @ M_g)[o]
over g in the 48 (filter k, sign) pairs, where phi_eff carries the
(-1)^s alternation for the minus branch.

Filter-dim sharding: 6 (k, sign) pairs per core; per-core partial
outputs are summed on host. All matmul operands are bf16 (f32 PSUM);
host-simulated end-to-end rel err ~4e-3 vs the 2e-2 budget.

Per-core structure (TS=128 sequence tiles, 16 of them):
  stage 1   Z[cp,pp,b][t,(p0 o|p1 o)] = x_b[cp]^T @ M_pp  (PE, bf16)
            evacuated to persistent SBUF bf16 tiles (DVE+ACT split)
  proj      W_p[cp][r,(b,o)] = P_p^T Z_p via col-tiled matmuls
            (tile_position strips; pairs 0-3 pack a 128-row A bank,
            pairs 4-5 a 64-row B bank), one evacuation copy per bank
  stage 2   per output tile c, ONE PSUM accumulation group [t,(b,o)]:
            far  (cp<=c-2): G_A^T @ W_A + G_B^T @ W_B   2 matmuls/cp
            near (d in 0,1): T_d,p^T @ Z_p,b            12 matmuls/cp
            (near matmuls write the b-half at a PSUM column offset)
            then one copy to SBUF staging and one DMA out.
G_{d,p} = P_p^T T_{d,p} with per-pair rank-32 bases (host eigh of the
far-block gram); rank 12 already suffices numerically.
"""

import numpy as np
import ml_dtypes

import concourse.bass as bass
import concourse.tile as tile
from concourse import mybir
from concourse.bass_utils import run_bass_kernel_spmd

L = 2048
K = 24
I = 256
O = 256
B = 2
TS = 128          # tile size along sequence
CT = L // TS      # 16 sequence tiles
NP = 6            # (k, sign) pairs per core
NPP = NP // 2     # fused pair-pairs
N_CORES = 8
RA = 32           # far-field rank, pairs 0-3 (32-strip aligned)
RB = 64           # far-field rank, pairs 4-5 (fills the B bank: full-row
                  # 128-part weight loads pipeline via the background buffer;
                  # 64-row loads serialize behind the in-flight matmul)
NA = 4 * RA       # A-bank rows
NB = 2 * RB       # B-bank rows
PBW = NA + NB     # fused basis columns
F32 = mybir.dt.float32
BF16 = mybir.dt.bfloat16


# ---------------------------------------------------------------------------
# Workarounds for this container's walrus: it rejects any instruction that
# carries more than one sync-wait command.
# ---------------------------------------------------------------------------

def _prune_init_barrier(nc):
    """Drop the Bass-init all-engine EVSEM barrier and the unused const
    memsets from the 'main' bb (~3us of EVSEM latency before any work)."""
    for f in nc.m.functions:
        for blk in f.blocks:
            if blk.name != "main":
                continue
            keep = []
            for inst in blk.instructions:
                nm = type(inst).__name__
                if nm in ("InstMemset", "InstDrain", "InstEventSemaphore"):
                    continue
                keep.append(inst)
            blk.instructions = keep


def _split_sync_waits(nc, max_waits=1):
    """Hoist extra sem-waits onto same-engine NOPs inserted right before the
    offending instruction; queue order keeps the semantics identical."""
    for f in nc.m.functions:
        for blk in f.blocks:
            insts = list(blk.instructions)
            out = []
            changed = False
            for inst in insts:
                si = getattr(inst, "sync_info", None)
                waits = list(si.on_wait) if si is not None else []
                if len(waits) > max_waits:
                    changed = True
                    extra, keep = waits[:-max_waits], waits[-max_waits:]
                    for j in range(0, len(extra), max_waits):
                        nop = mybir.InstNoOp(
                            name=nc.get_next_instruction_name(), ins=[], outs=[]
                        )
                        nop.engine = inst.engine
                        nop.sync_info = mybir.SyncInfo(
                            on_wait=extra[j : j + max_waits], on_update=[]
                        )
                        out.append(nop)
                    inst.sync_info = mybir.SyncInfo(
                        on_wait=keep, on_update=list(si.on_update)
                    )
                out.append(inst)
            if changed:
                blk.instructions = out


class _TC(tile.TileContext):
    """TileContext whose tail drain skips the global barrier (every DMA's
    completion is awaited by its consumer; nothing runs after this kernel)."""

    def _drain_and_barrier(self, tick_clock, wait_clock):
        nc = self.nc
        nc.sync.drain()
        assert self.sems is not None
        popped = nc._tile_sem_poison_stack.pop()
        assert popped is self._sem_poison


# ---------------------------------------------------------------------------
# Device program (identical on all 8 cores; per-core data differs)
# ---------------------------------------------------------------------------

def _build_nc():
    nc = bass.Bass("TRN2", target_bir_lowering=False, debug=False,
                   num_devices=N_CORES)
    # x batched per sequence tile: [cp, i', (b, ic, t)]
    xT_d = nc.dram_tensor("xT", [CT, TS, B * 2 * TS], BF16, kind="ExternalInput")
    # M fused per pair-pair: [pp, ic, i', (p0 o | p1 o)]
    m_d = nc.dram_tensor("m", [NPP, 2, TS, 2 * O], BF16, kind="ExternalInput")
    # dense Toeplitz blocks, diagonals 0..1: [d, t, (p, l)]
    tb_d = nc.dram_tensor("tb", [2, TS, NP * TS], BF16, kind="ExternalInput")
    # per-pair far bases, fused: [t, (p, r)]
    pb_d = nc.dram_tensor("pb", [TS, PBW], BF16, kind="ExternalInput")
    # far G stacks: A bank (pairs 0-3), B bank (pairs 4-5): [d-2, rows, l]
    gfa_d = nc.dram_tensor("gfa", [CT - 2, NA, TS], BF16, kind="ExternalInput")
    gfb_d = nc.dram_tensor("gfb", [CT - 2, NB, TS], BF16, kind="ExternalInput")
    # per-core partial output: [c, t, (b, o)]
    yp_d = nc.dram_tensor("yp", [CT, TS, B * O], F32, kind="ExternalOutput")

    with _TC(nc) as tc:
        with (
            tc.tile_pool(name="const", bufs=1) as cpool,
            tc.tile_pool(name="ystage", bufs=4) as ypool,
            tc.tile_pool(name="ps1", bufs=4, space="PSUM") as ps1,
            tc.tile_pool(name="psw", bufs=1, space="PSUM") as psw_pool,
            tc.tile_pool(name="psy", bufs=2, space="PSUM") as psy_pool,
        ):
            ms = [[cpool.tile([TS, 2 * O], BF16, tag=f"m{pp}{ic}",
                              name=f"m{pp}{ic}") for ic in range(2)]
                  for pp in range(NPP)]
            xs = [cpool.tile([TS, B * 2 * TS], BF16, tag=f"x{cp}",
                             name=f"x{cp}") for cp in range(CT)]
            tbs = [cpool.tile([TS, NP * TS], BF16, tag=f"t{d}", name=f"t{d}")
                   for d in range(2)]
            pbt = cpool.tile([TS, PBW], BF16, tag="pb", name="pbt")
            gta = [cpool.tile([NA, TS], BF16, tag=f"ga{d}", name=f"ga{d}")
                   for d in range(CT - 2)]
            gtb = [cpool.tile([NB, TS], BF16, tag=f"gb{d}", name=f"gb{d}")
                   for d in range(CT - 2)]
            zs = [[[cpool.tile([TS, 2 * O], BF16, tag=f"z{cp}_{pp}_{b}",
                               name=f"z{cp}_{pp}_{b}") for b in range(B)]
                   for pp in range(NPP)] for cp in range(CT)]
            wsa = [cpool.tile([NA, B * O], BF16, tag=f"wa{cp}",
                              name=f"wa{cp}") for cp in range(CT)]
            wsb = [cpool.tile([NB, B * O], BF16, tag=f"wb{cp}",
                              name=f"wb{cp}") for cp in range(CT)]

            # ---- head-critical DMAs (sync + gpsimd rings; scalar/vector
            # stay trigger-free for copies). Later xs/gf staggered in-loop.
            nc.sync.dma_start(ms[0][0][:], m_d[0, 0])
            nc.gpsimd.dma_start(xs[0][:], xT_d[0])
            nc.sync.dma_start(ms[0][1][:], m_d[0, 1])
            nc.gpsimd.dma_start(xs[1][:], xT_d[1])
            nc.sync.dma_start(ms[1][0][:], m_d[1, 0])
            nc.gpsimd.dma_start(ms[1][1][:], m_d[1, 1])
            nc.sync.dma_start(ms[2][0][:], m_d[2, 0])
            nc.gpsimd.dma_start(ms[2][1][:], m_d[2, 1])
            nc.gpsimd.dma_start(xs[2][:], xT_d[2])
            nc.sync.dma_start(tbs[0][:], tb_d[0])
            nc.gpsimd.dma_start(pbt[:], pb_d[:])
            nc.sync.dma_start(tbs[1][:], tb_d[1])

            cp_eng = [
                lambda o, i: nc.vector.tensor_copy(o, i),
                lambda o, i: nc.scalar.copy(o, i),
            ]

            def stage1(s):
                ci = s
                for pp in range(NPP):
                    for b in range(B):
                        ps = ps1.tile([TS, 2 * O], F32, tag="s1", name="ps")
                        for ic in range(2):
                            nc.tensor.matmul(
                                ps[:],
                                xs[s][:, (b * 2 + ic) * TS:(b * 2 + ic + 1) * TS],
                                ms[pp][ic][:],
                                start=(ic == 0),
                                stop=(ic == 1),
                            )
                        cp_eng[ci % 2](zs[s][pp][b][:, :O], ps[:, :O])
                        cp_eng[(ci + 1) % 2](zs[s][pp][b][:, O:], ps[:, O:])
                        ci += 1

            stage1(0)
            for s in range(CT):
                # prefetch: x two tiles ahead, far G for upcoming distances
                if s + 3 < CT:
                    nc.gpsimd.dma_start(xs[s + 3][:], xT_d[s + 3])
                if s < CT - 2:
                    nc.sync.dma_start(gta[s][:], gfa_d[s])
                    (nc.gpsimd if s % 2 else nc.sync).dma_start(
                        gtb[s][:], gfb_d[s])
                if s + 1 < CT:
                    stage1(s + 1)
                ci = s

                # ---- proj: W_A / W_B via col-tiled matmuls ----
                pwa = psw_pool.tile([NA, B * O], F32, tag="swA", name="pwa")
                for p in range(4):
                    for b in range(B):
                        nc.tensor.matmul(
                            pwa[RA * p:RA * (p + 1), b * O:(b + 1) * O],
                            pbt[:, p * RA:(p + 1) * RA],
                            zs[s][p // 2][b][:, (p % 2) * O:(p % 2 + 1) * O],
                            start=True, stop=True,
                            tile_position=(0, RA * p),
                        )
                pwb = psw_pool.tile([NB, B * O], F32, tag="swB", name="pwb")
                for p in (4, 5):
                    for b in range(B):
                        nc.tensor.matmul(
                            pwb[RB * (p - 4):RB * (p - 3), b * O:(b + 1) * O],
                            pbt[:, NA + (p - 4) * RB:NA + (p - 3) * RB],
                            zs[s][p // 2][b][:, (p % 2) * O:(p % 2 + 1) * O],
                            start=True, stop=True,
                            tile_position=(0, RB * (p - 4)),
                        )
                cp_eng[ci % 2](wsa[s][:], pwa[:]); ci += 1
                cp_eng[ci % 2](wsb[s][:], pwb[:]); ci += 1


                # ---- stage 2: output tile c = s, one PSUM group ----
                c = s
                yt = psy_pool.tile([TS, B * O], F32, tag="s2", name="yt")
                n_mm = 2 * max(0, c - 1) + 12 * (2 if c >= 1 else 1)
                i_mm = 0
                for cp in range(0, c - 1):  # far: d = c - cp >= 2
                    nc.tensor.matmul(
                        yt[:], gta[c - cp - 2][:], wsa[cp][:],
                        start=(i_mm == 0), stop=(i_mm == n_mm - 1),
                    )
                    i_mm += 1
                    nc.tensor.matmul(
                        yt[:], gtb[c - cp - 2][:], wsb[cp][:],
                        start=(i_mm == 0), stop=(i_mm == n_mm - 1),
                    )
                    i_mm += 1
                for cp in (c - 1, c):       # near: d in {1, 0}
                    if cp < 0:
                        continue
                    d = c - cp
                    for p in range(NP):
                        for b in range(B):
                            nc.tensor.matmul(
                                yt[:, b * O:(b + 1) * O],
                                tbs[d][:, p * TS:(p + 1) * TS],
                                zs[cp][p // 2][b][:, (p % 2) * O:(p % 2 + 1) * O],
                                start=(i_mm == 0), stop=(i_mm == n_mm - 1),
                            )
                            i_mm += 1
                assert i_mm == n_mm
                yst = ypool.tile([TS, B * O], F32, tag="yst", name=f"yst{c}")
                cp_eng[c % 2](yst[:], yt[:])
                (nc.sync if c % 2 else nc.gpsimd).dma_start(yp_d[c], yst[:])

    _prune_init_barrier(nc)
    _split_sync_waits(nc)
    return nc


# ---------------------------------------------------------------------------
# Host side: input staging, sharding, gather
# ---------------------------------------------------------------------------

def _build_toeplitz(phi_eff):
    """tb[d, t, l] = phi_eff[d*TS + l - t] (0 where the index is negative)."""
    pad = np.zeros(L + TS - 1, np.float32)
    pad[TS - 1:] = phi_eff
    d = np.arange(CT)[:, None, None]
    t = np.arange(TS)[None, :, None]
    l = np.arange(TS)[None, None, :]
    return pad[d * TS + l - t + TS - 1]


def _prepare(x, phi, M_phi_plus, M_phi_minus):
    """Host prep: build per-core in_maps (no device execution)."""
    bf = ml_dtypes.bfloat16
    x = np.asarray(x, np.float32)
    phi = np.asarray(phi, np.float32)
    Mp = np.asarray(M_phi_plus, np.float32)
    Mm = np.asarray(M_phi_minus, np.float32)

    # [cp, i', (b, ic, t)]: per-sequence-tile chunks of x^T, one DMA per cp
    xT = np.ascontiguousarray(
        x.reshape(B, CT, TS, 2, TS).transpose(1, 4, 0, 3, 2)
    ).reshape(CT, TS, B * 2 * TS).astype(bf)
    sgn = ((-1.0) ** np.arange(L)).astype(np.float32)

    m_all = np.empty((2 * K, 2, TS, O), np.float32)
    tb_all = np.empty((2 * K, CT, TS, TS), np.float32)
    for g in range(2 * K):
        k, sg = g // 2, g % 2
        m_all[g] = (Mm if sg else Mp)[k].reshape(2, TS, O)
        phi_eff = phi[:, k] * (sgn if sg else 1.0)
        tb_all[g] = _build_toeplitz(phi_eff)

    # per-pair far bases + projected G blocks (float64 host math).
    # Ranks: 32 for in-core pairs 0-3, 64 for pairs 4-5 (B bank fill).
    ranks = [32, 32, 32, 32, 64, 64]
    pb_all = [None] * (2 * K)
    gf_all = [None] * (2 * K)
    for g in range(2 * K):
        r = ranks[g % NP]
        far = tb_all[g, 2:].astype(np.float64)          # [14, t, l]
        gram = np.einsum('dtl,dsl->ts', far, far)
        _, vec = np.linalg.eigh(gram)
        P = np.ascontiguousarray(vec[:, ::-1][:, :r])   # [t, r]
        pb_all[g] = P
        gf_all[g] = np.einsum('tr,dtl->drl', P, far)    # [14, r, l]

    # fuse pair-pairs into columns: [pp, ic, i', (g0 o | g1 o)]
    m_fused = np.concatenate([m_all[0::2], m_all[1::2]], axis=3)

    nc = _build_nc()
    in_maps = []
    for core in range(N_CORES):
        gs = slice(core * NP, (core + 1) * NP)
        pps = slice(core * NPP, (core + 1) * NPP)
        tb_core = np.ascontiguousarray(
            tb_all[gs, :2].transpose(1, 2, 0, 3)
        ).reshape(2, TS, NP * TS).astype(bf)
        glist = [core * NP + j for j in range(NP)]
        pb_core = np.concatenate(
            [pb_all[g] for g in glist], axis=1).astype(bf)   # [t, PBW]
        gfa_core = np.concatenate(
            [gf_all[g] for g in glist[:4]], axis=1).astype(bf)  # [14, NA, l]
        gfb_core = np.concatenate(
            [gf_all[g] for g in glist[4:]], axis=1).astype(bf)  # [14, NB, l]
        in_maps.append({
            "xT": xT,
            "m": np.ascontiguousarray(m_fused[pps]).astype(bf),
            "tb": tb_core,
            "pb": pb_core,
            "gfa": gfa_core,
            "gfb": gfb_core,
        })
    return nc, in_maps


def _gather(results):
    y = np.zeros((CT, TS, B, O), np.float64)
    for core in range(N_CORES):
        y += results[core]["yp"].reshape(CT, TS, B, O).astype(np.float64)
    return np.ascontiguousarray(
        y.transpose(2, 0, 1, 3).reshape(B, L, O)
    ).astype(np.float32)


def kernel(x, phi, M_phi_plus, M_phi_minus):
    nc, in_maps = _prepare(x, phi, M_phi_plus, M_phi_minus)
    res = run_bass_kernel_spmd(nc, in_maps, list(range(N_CORES)))
    return _gather(res.results)


# revision 11
# speedup vs baseline: 1.0539x; 1.0539x over previous
"""MiniSTU Trainium2 kernel (8 NeuronCores, Bass/Tile).

Math: the reference's FFT convolution + einsum collapses to
    y[b,l,o] = sum_g sum_{t<=l} phi_eff_g[l-t] * (x[b,t] @ M_g)[o]
over g in the 48 (filter k, sign) pairs, where phi_eff carries the
(-1)^s alternation for the minus branch.

Filter-dim sharding: 6 (k, sign) pairs per core; per-core partial
outputs are summed on host. All matmul operands are bf16 (f32 PSUM);
host-simulated end-to-end rel err ~4e-3 vs the 2e-2 budget.

Per-core structure (TS=128 sequence tiles, 16 of them):
  stage 1   Z[cp,pp,b][t,(p0 o|p1 o)] = x_b[cp]^T @ M_pp  (PE, bf16)
            evacuated to persistent SBUF bf16 tiles (DVE+ACT split)
  proj      W_p[cp][r,(b,o)] = P_p^T Z_p via col-tiled matmuls
            (tile_position strips; pairs 0-3 pack a 128-row A bank,
            pairs 4-5 a 64-row B bank), one evacuation copy per bank
  stage 2   per output tile c, ONE PSUM accumulation group [t,(b,o)]:
            far  (cp<=c-2): G_A^T @ W_A + G_B^T @ W_B   2 matmuls/cp
            near (d in 0,1): T_d,p^T @ Z_p,b            12 matmuls/cp
            (near matmuls write the b-half at a PSUM column offset)
            then one copy to SBUF staging and one DMA out.
G_{d,p} = P_p^T T_{d,p} with per-pair rank-32 bases (host eigh of the
far-block gram); rank 12 already suffices numerically.
"""

import numpy as np
import ml_dtypes

import concourse.bass as bass
import concourse.tile as tile
from concourse import mybir
from concourse.bass_utils import run_bass_kernel_spmd

L = 2048
K = 24
I = 256
O = 256
B = 2
TS = 128          # tile size along sequence
CT = L // TS      # 16 sequence tiles
NP = 6            # (k, sign) pairs per core
NPP = NP // 2     # fused pair-pairs
N_CORES = 8
RA = 32           # far-field rank, pairs 0-3 (32-strip aligned)
RB = 64           # far-field rank, pairs 4-5 (fills the B bank: full-row
                  # 128-part weight loads pipeline via the background buffer;
                  # 64-row loads serialize behind the in-flight matmul)
NA = 4 * RA       # A-bank rows
NB = 2 * RB       # B-bank rows
PBW = NA + NB     # fused basis columns
F32 = mybir.dt.float32
BF16 = mybir.dt.bfloat16


# ---------------------------------------------------------------------------
# Workarounds for this container's walrus: it rejects any instruction that
# carries more than one sync-wait command.
# ---------------------------------------------------------------------------

def _prune_init_barrier(nc):
    """Drop the Bass-init all-engine EVSEM barrier and the unused const
    memsets from the 'main' bb (~3us of EVSEM latency before any work)."""
    for f in nc.m.functions:
        for blk in f.blocks:
            if blk.name != "main":
                continue
            keep = []
            for inst in blk.instructions:
                nm = type(inst).__name__
                if nm in ("InstMemset", "InstDrain", "InstEventSemaphore"):
                    continue
                keep.append(inst)
            blk.instructions = keep


def _split_sync_waits(nc, max_waits=1):
    """Hoist extra sem-waits onto same-engine NOPs inserted right before the
    offending instruction; queue order keeps the semantics identical."""
    for f in nc.m.functions:
        for blk in f.blocks:
            insts = list(blk.instructions)
            out = []
            changed = False
            for inst in insts:
                si = getattr(inst, "sync_info", None)
                waits = list(si.on_wait) if si is not None else []
                if len(waits) > max_waits:
                    changed = True
                    extra, keep = waits[:-max_waits], waits[-max_waits:]
                    for j in range(0, len(extra), max_waits):
                        nop = mybir.InstNoOp(
                            name=nc.get_next_instruction_name(), ins=[], outs=[]
                        )
                        nop.engine = inst.engine
                        nop.sync_info = mybir.SyncInfo(
                            on_wait=extra[j : j + max_waits], on_update=[]
                        )
                        out.append(nop)
                    inst.sync_info = mybir.SyncInfo(
                        on_wait=keep, on_update=list(si.on_update)
                    )
                out.append(inst)
            if changed:
                blk.instructions = out


class _TC(tile.TileContext):
    """TileContext whose tail drain skips the global barrier (every DMA's
    completion is awaited by its consumer; nothing runs after this kernel)."""

    def _drain_and_barrier(self, tick_clock, wait_clock):
        nc = self.nc
        nc.sync.drain()
        assert self.sems is not None
        popped = nc._tile_sem_poison_stack.pop()
        assert popped is self._sem_poison


# ---------------------------------------------------------------------------
# Device program (identical on all 8 cores; per-core data differs)
# ---------------------------------------------------------------------------

def _build_nc():
    nc = bass.Bass("TRN2", target_bir_lowering=False, debug=False,
                   num_devices=N_CORES)
    # x batched per sequence tile: [cp, i', (b, ic, t)]
    xT_d = nc.dram_tensor("xT", [CT, TS, B * 2 * TS], BF16, kind="ExternalInput")
    # M fused per pair-pair: [pp, ic, i', (p0 o | p1 o)]
    m_d = nc.dram_tensor("m", [NPP, 2, TS, 2 * O], BF16, kind="ExternalInput")
    # dense Toeplitz blocks, diagonals 0..1: [d, t, (p, l)]
    tb_d = nc.dram_tensor("tb", [2, TS, NP * TS], BF16, kind="ExternalInput")
    # per-pair far bases, fused: [t, (p, r)]
    pb_d = nc.dram_tensor("pb", [TS, PBW], BF16, kind="ExternalInput")
    # far G stacks: A bank (pairs 0-3), B bank (pairs 4-5): [d-2, rows, l]
    gfa_d = nc.dram_tensor("gfa", [CT - 2, NA, TS], BF16, kind="ExternalInput")
    gfb_d = nc.dram_tensor("gfb", [CT - 2, NB, TS], BF16, kind="ExternalInput")
    # per-core partial output: [c, t, (b, o)]
    yp_d = nc.dram_tensor("yp", [CT, TS, B * O], F32, kind="ExternalOutput")

    with _TC(nc) as tc:
        with (
            tc.tile_pool(name="const", bufs=1) as cpool,
            tc.tile_pool(name="ystage", bufs=4) as ypool,
            tc.tile_pool(name="ps1", bufs=4, space="PSUM") as ps1,
            tc.tile_pool(name="psw", bufs=1, space="PSUM") as psw_pool,
            tc.tile_pool(name="psy", bufs=2, space="PSUM") as psy_pool,
        ):
            ms = [[cpool.tile([TS, 2 * O], BF16, tag=f"m{pp}{ic}",
                              name=f"m{pp}{ic}") for ic in range(2)]
                  for pp in range(NPP)]
            xs = [cpool.tile([TS, B * 2 * TS], BF16, tag=f"x{cp}",
                             name=f"x{cp}") for cp in range(CT)]
            tbs = [cpool.tile([TS, NP * TS], BF16, tag=f"t{d}", name=f"t{d}")
                   for d in range(2)]
            pbt = cpool.tile([TS, PBW], BF16, tag="pb", name="pbt")
            gta = [cpool.tile([NA, TS], BF16, tag=f"ga{d}", name=f"ga{d}")
                   for d in range(CT - 2)]
            gtb = [cpool.tile([NB, TS], BF16, tag=f"gb{d}", name=f"gb{d}")
                   for d in range(CT - 2)]
            zs = [[[cpool.tile([TS, 2 * O], BF16, tag=f"z{cp}_{pp}_{b}",
                               name=f"z{cp}_{pp}_{b}") for b in range(B)]
                   for pp in range(NPP)] for cp in range(CT)]
            wsa = [cpool.tile([NA, B * O], BF16, tag=f"wa{cp}",
                              name=f"wa{cp}") for cp in range(CT)]
            wsb = [cpool.tile([NB, B * O], BF16, tag=f"wb{cp}",
                              name=f"wb{cp}") for cp in range(CT)]

            # ---- head-critical DMAs (sync + gpsimd rings; scalar/vector
            # stay trigger-free for copies). Later xs/gf staggered in-loop.
            nc.sync.dma_start(ms[0][0][:], m_d[0, 0])
            nc.gpsimd.dma_start(xs[0][:], xT_d[0])
            nc.sync.dma_start(ms[0][1][:], m_d[0, 1])
            nc.gpsimd.dma_start(xs[1][:], xT_d[1])
            nc.sync.dma_start(ms[1][0][:], m_d[1, 0])
            nc.gpsimd.dma_start(ms[1][1][:], m_d[1, 1])
            nc.sync.dma_start(ms[2][0][:], m_d[2, 0])
            nc.gpsimd.dma_start(ms[2][1][:], m_d[2, 1])
            nc.gpsimd.dma_start(xs[2][:], xT_d[2])
            nc.sync.dma_start(tbs[0][:], tb_d[0])
            nc.gpsimd.dma_start(pbt[:], pb_d[:])
            nc.sync.dma_start(tbs[1][:], tb_d[1])

            cp_eng = [
                lambda o, i: nc.vector.tensor_copy(o, i),
                lambda o, i: nc.scalar.copy(o, i),
            ]

            def stage1(s):
                ci = s
                for pp in range(NPP):
                    for b in range(B):
                        ps = ps1.tile([TS, 2 * O], F32, tag="s1", name="ps")
                        for ic in range(2):
                            nc.tensor.matmul(
                                ps[:],
                                xs[s][:, (b * 2 + ic) * TS:(b * 2 + ic + 1) * TS],
                                ms[pp][ic][:],
                                start=(ic == 0),
                                stop=(ic == 1),
                            )
                        cp_eng[ci % 2](zs[s][pp][b][:], ps[:])
                        ci += 1

            stage1(0)
            for s in range(CT):
                # prefetch: x two tiles ahead, far G for upcoming distances
                if s + 3 < CT:
                    nc.gpsimd.dma_start(xs[s + 3][:], xT_d[s + 3])
                if s < CT - 2:
                    nc.sync.dma_start(gta[s][:], gfa_d[s])
                    (nc.gpsimd if s % 2 else nc.sync).dma_start(
                        gtb[s][:], gfb_d[s])
                if s + 1 < CT:
                    stage1(s + 1)
                ci = s

                # ---- proj: W_A / W_B via col-tiled matmuls ----
                pwa = psw_pool.tile([NA, B * O], F32, tag="swA", name="pwa")
                for p in range(4):
                    for b in range(B):
                        nc.tensor.matmul(
                            pwa[RA * p:RA * (p + 1), b * O:(b + 1) * O],
                            pbt[:, p * RA:(p + 1) * RA],
                            zs[s][p // 2][b][:, (p % 2) * O:(p % 2 + 1) * O],
                            start=True, stop=True,
                            tile_position=(0, RA * p),
                        )
                pwb = psw_pool.tile([NB, B * O], F32, tag="swB", name="pwb")
                for p in (4, 5):
                    for b in range(B):
                        nc.tensor.matmul(
                            pwb[RB * (p - 4):RB * (p - 3), b * O:(b + 1) * O],
                            pbt[:, NA + (p - 4) * RB:NA + (p - 3) * RB],
                            zs[s][p // 2][b][:, (p % 2) * O:(p % 2 + 1) * O],
                            start=True, stop=True,
                            tile_position=(0, RB * (p - 4)),
                        )
                cp_eng[ci % 2](wsa[s][:], pwa[:]); ci += 1
                cp_eng[ci % 2](wsb[s][:], pwb[:]); ci += 1


                # ---- stage 2: output tile c = s, one PSUM group ----
                c = s
                yt = psy_pool.tile([TS, B * O], F32, tag="s2", name="yt")
                n_mm = 2 * max(0, c - 1) + 12 * (2 if c >= 1 else 1)
                i_mm = 0
                for cp in range(0, c - 1):  # far: d = c - cp >= 2
                    nc.tensor.matmul(
                        yt[:], gta[c - cp - 2][:], wsa[cp][:],
                        start=(i_mm == 0), stop=(i_mm == n_mm - 1),
                    )
                    i_mm += 1
                    nc.tensor.matmul(
                        yt[:], gtb[c - cp - 2][:], wsb[cp][:],
                        start=(i_mm == 0), stop=(i_mm == n_mm - 1),
                    )
                    i_mm += 1
                for cp in (c - 1, c):       # near: d in {1, 0}
                    if cp < 0:
                        continue
                    d = c - cp
                    for p in range(NP):
                        for b in range(B):
                            nc.tensor.matmul(
                                yt[:, b * O:(b + 1) * O],
                                tbs[d][:, p * TS:(p + 1) * TS],
                                zs[cp][p // 2][b][:, (p % 2) * O:(p % 2 + 1) * O],
                                start=(i_mm == 0), stop=(i_mm == n_mm - 1),
                            )
                            i_mm += 1
                assert i_mm == n_mm
                yst = ypool.tile([TS, B * O], F32, tag="yst", name=f"yst{c}")
                cp_eng[c % 2](yst[:], yt[:])
                (nc.sync if c % 2 else nc.gpsimd).dma_start(yp_d[c], yst[:])

    _prune_init_barrier(nc)
    _split_sync_waits(nc)
    return nc


# ---------------------------------------------------------------------------
# Host side: input staging, sharding, gather
# ---------------------------------------------------------------------------

def _build_toeplitz(phi_eff):
    """tb[d, t, l] = phi_eff[d*TS + l - t] (0 where the index is negative)."""
    pad = np.zeros(L + TS - 1, np.float32)
    pad[TS - 1:] = phi_eff
    d = np.arange(CT)[:, None, None]
    t = np.arange(TS)[None, :, None]
    l = np.arange(TS)[None, None, :]
    return pad[d * TS + l - t + TS - 1]


def _prepare(x, phi, M_phi_plus, M_phi_minus):
    """Host prep: build per-core in_maps (no device execution)."""
    bf = ml_dtypes.bfloat16
    x = np.asarray(x, np.float32)
    phi = np.asarray(phi, np.float32)
    Mp = np.asarray(M_phi_plus, np.float32)
    Mm = np.asarray(M_phi_minus, np.float32)

    # [cp, i', (b, ic, t)]: per-sequence-tile chunks of x^T, one DMA per cp
    xT = np.ascontiguousarray(
        x.reshape(B, CT, TS, 2, TS).transpose(1, 4, 0, 3, 2)
    ).reshape(CT, TS, B * 2 * TS).astype(bf)
    sgn = ((-1.0) ** np.arange(L)).astype(np.float32)

    m_all = np.empty((2 * K, 2, TS, O), np.float32)
    tb_all = np.empty((2 * K, CT, TS, TS), np.float32)
    for g in range(2 * K):
        k, sg = g // 2, g % 2
        m_all[g] = (Mm if sg else Mp)[k].reshape(2, TS, O)
        phi_eff = phi[:, k] * (sgn if sg else 1.0)
        tb_all[g] = _build_toeplitz(phi_eff)

    # per-pair far bases + projected G blocks (float64 host math).
    # Ranks: 32 for in-core pairs 0-3, 64 for pairs 4-5 (B bank fill).
    ranks = [32, 32, 32, 32, 64, 64]
    pb_all = [None] * (2 * K)
    gf_all = [None] * (2 * K)
    for g in range(2 * K):
        r = ranks[g % NP]
        far = tb_all[g, 2:].astype(np.float64)          # [14, t, l]
        gram = np.einsum('dtl,dsl->ts', far, far)
        _, vec = np.linalg.eigh(gram)
        P = np.ascontiguousarray(vec[:, ::-1][:, :r])   # [t, r]
        pb_all[g] = P
        gf_all[g] = np.einsum('tr,dtl->drl', P, far)    # [14, r, l]

    # fuse pair-pairs into columns: [pp, ic, i', (g0 o | g1 o)]
    m_fused = np.concatenate([m_all[0::2], m_all[1::2]], axis=3)

    nc = _build_nc()
    in_maps = []
    for core in range(N_CORES):
        gs = slice(core * NP, (core + 1) * NP)
        pps = slice(core * NPP, (core + 1) * NPP)
        tb_core = np.ascontiguousarray(
            tb_all[gs, :2].transpose(1, 2, 0, 3)
        ).reshape(2, TS, NP * TS).astype(bf)
        glist = [core * NP + j for j in range(NP)]
        pb_core = np.concatenate(
            [pb_all[g] for g in glist], axis=1).astype(bf)   # [t, PBW]
        gfa_core = np.concatenate(
            [gf_all[g] for g in glist[:4]], axis=1).astype(bf)  # [14, NA, l]
        gfb_core = np.concatenate(
            [gf_all[g] for g in glist[4:]], axis=1).astype(bf)  # [14, NB, l]
        in_maps.append({
            "xT": xT,
            "m": np.ascontiguousarray(m_fused[pps]).astype(bf),
            "tb": tb_core,
            "pb": pb_core,
            "gfa": gfa_core,
            "gfb": gfb_core,
        })
    return nc, in_maps


def _gather(results):
    y = np.zeros((CT, TS, B, O), np.float64)
    for core in range(N_CORES):
        y += results[core]["yp"].reshape(CT, TS, B, O).astype(np.float64)
    return np.ascontiguousarray(
        y.transpose(2, 0, 1, 3).reshape(B, L, O)
    ).astype(np.float32)


def kernel(x, phi, M_phi_plus, M_phi_minus):
    nc, in_maps = _prepare(x, phi, M_phi_plus, M_phi_minus)
    res = run_bass_kernel_spmd(nc, in_maps, list(range(N_CORES)))
    return _gather(res.results)


# revision 12
# speedup vs baseline: 1.0550x; 1.0011x over previous
"""MiniSTU Trainium2 kernel (8 NeuronCores, Bass/Tile).

Math: the reference's FFT convolution + einsum collapses to
    y[b,l,o] = sum_g sum_{t<=l} phi_eff_g[l-t] * (x[b,t] @ M_g)[o]
over g in the 48 (filter k, sign) pairs, where phi_eff carries the
(-1)^s alternation for the minus branch.

Filter-dim sharding: 6 (k, sign) pairs per core; per-core partial
outputs are summed on host. All matmul operands are bf16 (f32 PSUM);
host-simulated end-to-end rel err ~4e-3 vs the 2e-2 budget.

Per-core structure (TS=128 sequence tiles, 16 of them):
  stage 1   Z[cp,pp,b][t,(p0 o|p1 o)] = x_b[cp]^T @ M_pp  (PE, bf16)
            evacuated to persistent SBUF bf16 tiles (DVE+ACT split)
  proj      W_p[cp][r,(b,o)] = P_p^T Z_p via col-tiled matmuls
            (tile_position strips; pairs 0-3 pack a 128-row A bank,
            pairs 4-5 a 64-row B bank), one evacuation copy per bank
  stage 2   per output tile c, ONE PSUM accumulation group [t,(b,o)]:
            far  (cp<=c-2): G_A^T @ W_A + G_B^T @ W_B   2 matmuls/cp
            near (d in 0,1): T_d,p^T @ Z_p,b            12 matmuls/cp
            (near matmuls write the b-half at a PSUM column offset)
            then one copy to SBUF staging and one DMA out.
G_{d,p} = P_p^T T_{d,p} with per-pair rank-32 bases (host eigh of the
far-block gram); rank 12 already suffices numerically.
"""

import numpy as np
import ml_dtypes

import concourse.bass as bass
import concourse.tile as tile
from concourse import mybir
from concourse.bass_utils import run_bass_kernel_spmd

L = 2048
K = 24
I = 256
O = 256
B = 2
TS = 128          # tile size along sequence
CT = L // TS      # 16 sequence tiles
NP = 6            # (k, sign) pairs per core
NPP = NP // 2     # fused pair-pairs
N_CORES = 8
RA = 32           # far-field rank, pairs 0-3 (32-strip aligned)
RB = 64           # far-field rank, pairs 4-5 (fills the B bank: full-row
                  # 128-part weight loads pipeline via the background buffer;
                  # 64-row loads serialize behind the in-flight matmul)
NA = 4 * RA       # A-bank rows
NB = 2 * RB       # B-bank rows
PBW = NA + NB     # fused basis columns
F32 = mybir.dt.float32
BF16 = mybir.dt.bfloat16


# ---------------------------------------------------------------------------
# Workarounds for this container's walrus: it rejects any instruction that
# carries more than one sync-wait command.
# ---------------------------------------------------------------------------

def _prune_init_barrier(nc):
    """Drop the Bass-init all-engine EVSEM barrier and the unused const
    memsets from the 'main' bb (~3us of EVSEM latency before any work)."""
    for f in nc.m.functions:
        for blk in f.blocks:
            if blk.name != "main":
                continue
            keep = []
            for inst in blk.instructions:
                nm = type(inst).__name__
                if nm in ("InstMemset", "InstDrain", "InstEventSemaphore"):
                    continue
                keep.append(inst)
            blk.instructions = keep


def _split_sync_waits(nc, max_waits=1):
    """Hoist extra sem-waits onto same-engine NOPs inserted right before the
    offending instruction; queue order keeps the semantics identical."""
    for f in nc.m.functions:
        for blk in f.blocks:
            insts = list(blk.instructions)
            out = []
            changed = False
            for inst in insts:
                si = getattr(inst, "sync_info", None)
                waits = list(si.on_wait) if si is not None else []
                if len(waits) > max_waits:
                    changed = True
                    extra, keep = waits[:-max_waits], waits[-max_waits:]
                    for j in range(0, len(extra), max_waits):
                        nop = mybir.InstNoOp(
                            name=nc.get_next_instruction_name(), ins=[], outs=[]
                        )
                        nop.engine = inst.engine
                        nop.sync_info = mybir.SyncInfo(
                            on_wait=extra[j : j + max_waits], on_update=[]
                        )
                        out.append(nop)
                    inst.sync_info = mybir.SyncInfo(
                        on_wait=keep, on_update=list(si.on_update)
                    )
                out.append(inst)
            if changed:
                blk.instructions = out


class _TC(tile.TileContext):
    """TileContext whose tail drain skips the global barrier (every DMA's
    completion is awaited by its consumer; nothing runs after this kernel)."""

    def _drain_and_barrier(self, tick_clock, wait_clock):
        nc = self.nc
        nc.sync.drain()
        assert self.sems is not None
        popped = nc._tile_sem_poison_stack.pop()
        assert popped is self._sem_poison


# ---------------------------------------------------------------------------
# Device program (identical on all 8 cores; per-core data differs)
# ---------------------------------------------------------------------------

def _build_nc():
    nc = bass.Bass("TRN2", target_bir_lowering=False, debug=False,
                   num_devices=N_CORES)
    # x batched per sequence tile: [cp, i', (b, ic, t)]
    xT_d = nc.dram_tensor("xT", [CT, TS, B * 2 * TS], BF16, kind="ExternalInput")
    # M fused per pair-pair: [pp, ic, i', (p0 o | p1 o)]
    m_d = nc.dram_tensor("m", [NPP, 2, TS, 2 * O], BF16, kind="ExternalInput")
    # dense Toeplitz blocks, diagonals 0..1: [d, t, (p, l)]
    tb_d = nc.dram_tensor("tb", [2, TS, NP * TS], BF16, kind="ExternalInput")
    # per-pair far bases, fused: [t, (p, r)]
    pb_d = nc.dram_tensor("pb", [TS, PBW], BF16, kind="ExternalInput")
    # far G stacks: A bank (pairs 0-3), B bank (pairs 4-5): [d-2, rows, l]
    gfa_d = nc.dram_tensor("gfa", [CT - 2, NA, TS], BF16, kind="ExternalInput")
    gfb_d = nc.dram_tensor("gfb", [CT - 2, NB, TS], BF16, kind="ExternalInput")
    # per-core partial output: [c, t, (b, o)]
    yp_d = nc.dram_tensor("yp", [CT, TS, B * O], F32, kind="ExternalOutput")

    with _TC(nc) as tc:
        with (
            tc.tile_pool(name="const", bufs=1) as cpool,
            tc.tile_pool(name="ystage", bufs=4) as ypool,
            tc.tile_pool(name="ps1", bufs=6, space="PSUM") as ps1,
            tc.tile_pool(name="psy", bufs=2, space="PSUM") as psy_pool,
        ):
            ms = [[cpool.tile([TS, 2 * O], BF16, tag=f"m{pp}{ic}",
                              name=f"m{pp}{ic}") for ic in range(2)]
                  for pp in range(NPP)]
            xs = [cpool.tile([TS, B * 2 * TS], BF16, tag=f"x{cp}",
                             name=f"x{cp}") for cp in range(CT)]
            tbs = [cpool.tile([TS, NP * TS], BF16, tag=f"t{d}", name=f"t{d}")
                   for d in range(2)]
            pbt = cpool.tile([TS, PBW], BF16, tag="pb", name="pbt")
            gta = [cpool.tile([NA, TS], BF16, tag=f"ga{d}", name=f"ga{d}")
                   for d in range(CT - 2)]
            gtb = [cpool.tile([NB, TS], BF16, tag=f"gb{d}", name=f"gb{d}")
                   for d in range(CT - 2)]
            zs = [[[cpool.tile([TS, 2 * O], BF16, tag=f"z{cp}_{pp}_{b}",
                               name=f"z{cp}_{pp}_{b}") for b in range(B)]
                   for pp in range(NPP)] for cp in range(CT)]
            wsa = [cpool.tile([NA, B * O], BF16, tag=f"wa{cp}",
                              name=f"wa{cp}") for cp in range(CT)]
            wsb = [cpool.tile([NB, B * O], BF16, tag=f"wb{cp}",
                              name=f"wb{cp}") for cp in range(CT)]

            # ---- head-critical DMAs (sync + gpsimd rings; scalar/vector
            # stay trigger-free for copies). Later xs/gf staggered in-loop.
            nc.sync.dma_start(ms[0][0][:], m_d[0, 0])
            nc.gpsimd.dma_start(xs[0][:], xT_d[0])
            nc.sync.dma_start(ms[0][1][:], m_d[0, 1])
            nc.gpsimd.dma_start(xs[1][:], xT_d[1])
            nc.sync.dma_start(ms[1][0][:], m_d[1, 0])
            nc.gpsimd.dma_start(ms[1][1][:], m_d[1, 1])
            nc.sync.dma_start(ms[2][0][:], m_d[2, 0])
            nc.gpsimd.dma_start(ms[2][1][:], m_d[2, 1])
            nc.gpsimd.dma_start(xs[2][:], xT_d[2])
            nc.sync.dma_start(tbs[0][:], tb_d[0])
            nc.gpsimd.dma_start(pbt[:], pb_d[:])
            nc.sync.dma_start(tbs[1][:], tb_d[1])

            cp_eng = [
                lambda o, i: nc.vector.tensor_copy(o, i),
                lambda o, i: nc.scalar.copy(o, i),
            ]

            def stage1(s):
                ci = s
                for pp in range(NPP):
                    for b in range(B):
                        ps = ps1.tile([TS, 2 * O], F32, tag="s1", name="ps")
                        for ic in range(2):
                            nc.tensor.matmul(
                                ps[:],
                                xs[s][:, (b * 2 + ic) * TS:(b * 2 + ic + 1) * TS],
                                ms[pp][ic][:],
                                start=(ic == 0),
                                stop=(ic == 1),
                            )
                        cp_eng[ci % 2](zs[s][pp][b][:], ps[:])
                        ci += 1

            stage1(0)
            for s in range(CT):
                # prefetch: x two tiles ahead, far G for upcoming distances
                if s + 3 < CT:
                    nc.gpsimd.dma_start(xs[s + 3][:], xT_d[s + 3])
                if s < CT - 2:
                    nc.sync.dma_start(gta[s][:], gfa_d[s])
                    (nc.gpsimd if s % 2 else nc.sync).dma_start(
                        gtb[s][:], gfb_d[s])
                if s + 1 < CT:
                    stage1(s + 1)
                ci = s

                # ---- proj: W_A / W_B via col-tiled matmuls ----
                pwa = ps1.tile([NA, B * O], F32, tag="s1", name="pwa")
                for p in range(4):
                    for b in range(B):
                        nc.tensor.matmul(
                            pwa[RA * p:RA * (p + 1), b * O:(b + 1) * O],
                            pbt[:, p * RA:(p + 1) * RA],
                            zs[s][p // 2][b][:, (p % 2) * O:(p % 2 + 1) * O],
                            start=True, stop=True,
                            tile_position=(0, RA * p),
                        )
                pwb = ps1.tile([NB, B * O], F32, tag="s1", name="pwb")
                for p in (4, 5):
                    for b in range(B):
                        nc.tensor.matmul(
                            pwb[RB * (p - 4):RB * (p - 3), b * O:(b + 1) * O],
                            pbt[:, NA + (p - 4) * RB:NA + (p - 3) * RB],
                            zs[s][p // 2][b][:, (p % 2) * O:(p % 2 + 1) * O],
                            start=True, stop=True,
                            tile_position=(0, RB * (p - 4)),
                        )
                cp_eng[ci % 2](wsa[s][:], pwa[:]); ci += 1
                cp_eng[ci % 2](wsb[s][:], pwb[:]); ci += 1


                # ---- stage 2: output tile c = s, one PSUM group ----
                c = s
                yt = psy_pool.tile([TS, B * O], F32, tag="s2", name="yt")
                n_mm = 2 * max(0, c - 1) + 12 * (2 if c >= 1 else 1)
                i_mm = 0
                for cp in range(0, c - 1):  # far: d = c - cp >= 2
                    nc.tensor.matmul(
                        yt[:], gta[c - cp - 2][:], wsa[cp][:],
                        start=(i_mm == 0), stop=(i_mm == n_mm - 1),
                    )
                    i_mm += 1
                    nc.tensor.matmul(
                        yt[:], gtb[c - cp - 2][:], wsb[cp][:],
                        start=(i_mm == 0), stop=(i_mm == n_mm - 1),
                    )
                    i_mm += 1
                for cp in (c - 1, c):       # near: d in {1, 0}
                    if cp < 0:
                        continue
                    d = c - cp
                    for p in range(NP):
                        for b in range(B):
                            nc.tensor.matmul(
                                yt[:, b * O:(b + 1) * O],
                                tbs[d][:, p * TS:(p + 1) * TS],
                                zs[cp][p // 2][b][:, (p % 2) * O:(p % 2 + 1) * O],
                                start=(i_mm == 0), stop=(i_mm == n_mm - 1),
                            )
                            i_mm += 1
                assert i_mm == n_mm
                yst = ypool.tile([TS, B * O], F32, tag="yst", name=f"yst{c}")
                cp_eng[c % 2](yst[:], yt[:])
                (nc.sync if c % 2 else nc.gpsimd).dma_start(yp_d[c], yst[:])

    _prune_init_barrier(nc)
    _split_sync_waits(nc)
    return nc


# ---------------------------------------------------------------------------
# Host side: input staging, sharding, gather
# ---------------------------------------------------------------------------

def _build_toeplitz(phi_eff):
    """tb[d, t, l] = phi_eff[d*TS + l - t] (0 where the index is negative)."""
    pad = np.zeros(L + TS - 1, np.float32)
    pad[TS - 1:] = phi_eff
    d = np.arange(CT)[:, None, None]
    t = np.arange(TS)[None, :, None]
    l = np.arange(TS)[None, None, :]
    return pad[d * TS + l - t + TS - 1]


def _prepare(x, phi, M_phi_plus, M_phi_minus):
    """Host prep: build per-core in_maps (no device execution)."""
    bf = ml_dtypes.bfloat16
    x = np.asarray(x, np.float32)
    phi = np.asarray(phi, np.float32)
    Mp = np.asarray(M_phi_plus, np.float32)
    Mm = np.asarray(M_phi_minus, np.float32)

    # [cp, i', (b, ic, t)]: per-sequence-tile chunks of x^T, one DMA per cp
    xT = np.ascontiguousarray(
        x.reshape(B, CT, TS, 2, TS).transpose(1, 4, 0, 3, 2)
    ).reshape(CT, TS, B * 2 * TS).astype(bf)
    sgn = ((-1.0) ** np.arange(L)).astype(np.float32)

    m_all = np.empty((2 * K, 2, TS, O), np.float32)
    tb_all = np.empty((2 * K, CT, TS, TS), np.float32)
    for g in range(2 * K):
        k, sg = g // 2, g % 2
        m_all[g] = (Mm if sg else Mp)[k].reshape(2, TS, O)
        phi_eff = phi[:, k] * (sgn if sg else 1.0)
        tb_all[g] = _build_toeplitz(phi_eff)

    # per-pair far bases + projected G blocks (float64 host math).
    # Ranks: 32 for in-core pairs 0-3, 64 for pairs 4-5 (B bank fill).
    ranks = [32, 32, 32, 32, 64, 64]
    pb_all = [None] * (2 * K)
    gf_all = [None] * (2 * K)
    for g in range(2 * K):
        r = ranks[g % NP]
        far = tb_all[g, 2:].astype(np.float64)          # [14, t, l]
        gram = np.einsum('dtl,dsl->ts', far, far)
        _, vec = np.linalg.eigh(gram)
        P = np.ascontiguousarray(vec[:, ::-1][:, :r])   # [t, r]
        pb_all[g] = P
        gf_all[g] = np.einsum('tr,dtl->drl', P, far)    # [14, r, l]

    # fuse pair-pairs into columns: [pp, ic, i', (g0 o | g1 o)]
    m_fused = np.concatenate([m_all[0::2], m_all[1::2]], axis=3)

    nc = _build_nc()
    in_maps = []
    for core in range(N_CORES):
        gs = slice(core * NP, (core + 1) * NP)
        pps = slice(core * NPP, (core + 1) * NPP)
        tb_core = np.ascontiguousarray(
            tb_all[gs, :2].transpose(1, 2, 0, 3)
        ).reshape(2, TS, NP * TS).astype(bf)
        glist = [core * NP + j for j in range(NP)]
        pb_core = np.concatenate(
            [pb_all[g] for g in glist], axis=1).astype(bf)   # [t, PBW]
        gfa_core = np.concatenate(
            [gf_all[g] for g in glist[:4]], axis=1).astype(bf)  # [14, NA, l]
        gfb_core = np.concatenate(
            [gf_all[g] for g in glist[4:]], axis=1).astype(bf)  # [14, NB, l]
        in_maps.append({
            "xT": xT,
            "m": np.ascontiguousarray(m_fused[pps]).astype(bf),
            "tb": tb_core,
            "pb": pb_core,
            "gfa": gfa_core,
            "gfb": gfb_core,
        })
    return nc, in_maps


def _gather(results):
    y = np.zeros((CT, TS, B, O), np.float64)
    for core in range(N_CORES):
        y += results[core]["yp"].reshape(CT, TS, B, O).astype(np.float64)
    return np.ascontiguousarray(
        y.transpose(2, 0, 1, 3).reshape(B, L, O)
    ).astype(np.float32)


def kernel(x, phi, M_phi_plus, M_phi_minus):
    nc, in_maps = _prepare(x, phi, M_phi_plus, M_phi_minus)
    res = run_bass_kernel_spmd(nc, in_maps, list(range(N_CORES)))
    return _gather(res.results)
